# revision 19
# baseline (speedup 1.0000x reference)
"""CoAtten2 Trainium2 kernel: 8-way tensor-parallel over one TRN2 chip.

Reference computation (C=1024, H=W=64, HW=4096):
    q   = (Wq @ Xm + bq)  viewed [1024, 2048] then transposed
    kf  = (Wk1 @ Xf + bk1) viewed [1024, 2048]
    kl  = (Wk2 @ Xl + bk2) viewed [1024, 2048]
    att = softmax(kf @ q) + softmax(kl @ q)          # [1024, 1024]
    out = gamma * (att @ (Wv @ Xm + bv)) + (Xf + Xl)/2

Decomposition (per core d of 8; group t = d//4, a = d%4):
  - Channel indices are permuted (I' = 512t + o <-> i = 2o + t) so the
    torch-style reshape becomes contiguous; the permutation is folded into the
    host-side Wv/bv prep and the output DMA access pattern; gamma into Wv/bv.
  - logits_PERM splits into parity quadrants Q(t, t') whose kf operand needs
    spatial columns [2048t, 2048t+2048) and whose q operand needs spatial
    columns [2048t', ...). Core d owns spatial slice S_d = [512d, 512(d+1));
    it computes the partial contraction over S_d of Q(t=d//4, t'=0 and 1) for
    both attention branches, using ONLY local projections:
      CkfT_d / CklT_d from its own x slices, and CqT over spatial blocks
      (d%4) and (4 + d%4) from two host-provided Xm slices (xq0/xq1).
  - Per branch, one 4-way fp16 ReduceScatter (groups [0-3], [4-7]) sums the
    partial logits and deals each core its 128-row attention block.
  - softmax is a free-dim reduction; the summed attention is AllGathered in
    fp16; att^T j-chunk k feeds the final att @ V_d matmul as soon as its
    transpose-DMA lands, pipelining the transposes with the matmul.
  - All data-plane tensors are fp16 (x, weights, partials, att, v); f32 is
    kept for PSUM accumulation, softmax internals, biases, residual and the
    final output.
"""

import sys

sys.path.insert(0, "/opt/trn_rl_repo")

import numpy as np

import concourse.bacc as bacc
import concourse.mybir as mybir
from concourse import tile
from concourse.bass_utils import run_bass_kernel_spmd

F32 = mybir.dt.float32
F16 = mybir.dt.float16

C = 1024
HW = 4096
S = 512          # spatial columns per core
CH = 512         # C // 2 (projection output channels)
NCORES = 8

_CACHE: dict = {}


def _build():
    nc = bacc.Bacc("TRN2", target_bir_lowering=False, debug=False, num_devices=NCORES)

    # per-core external inputs (fp16 data plane, f32 biases)
    xm = nc.declare_dram_parameter("xm", [C, S], F16, isOutput=False)
    xf = nc.declare_dram_parameter("xf", [C, S], F16, isOutput=False)
    xl = nc.declare_dram_parameter("xl", [C, S], F16, isOutput=False)
    xq0 = nc.declare_dram_parameter("xq0", [C, S], F16, isOutput=False)  # Xm block d%4
    xq1 = nc.declare_dram_parameter("xq1", [C, S], F16, isOutput=False)  # Xm block 4+d%4
    wq = nc.declare_dram_parameter("wq", [C, CH], F16, isOutput=False)   # Wq.T
    wk1 = nc.declare_dram_parameter("wk1", [C, CH], F16, isOutput=False)
    wk2 = nc.declare_dram_parameter("wk2", [C, CH], F16, isOutput=False)
    wv = nc.declare_dram_parameter("wv", [C, C], F16, isOutput=False)    # (g*Wv)[permJ].T
    bqr = nc.declare_dram_parameter("bqr", [128, CH], F32, isOutput=False)
    bk1r = nc.declare_dram_parameter("bk1r", [128, CH], F32, isOutput=False)
    bk2r = nc.declare_dram_parameter("bk2r", [128, CH], F32, isOutput=False)
    bvp = nc.declare_dram_parameter("bvp", [128, 8], F32, isOutput=False)
    out_ext = nc.declare_dram_parameter("out", [C, S], F32, isOutput=True)

    # internal DRAM
    rs_f_in = nc.dram_tensor("rs_f_in", [CH, C], F16)
    rs_f_out = nc.dram_tensor("rs_f_out", [128, C], F16)
    rs_l_in = nc.dram_tensor("rs_l_in", [CH, C], F16)
    rs_l_out = nc.dram_tensor("rs_l_out", [128, C], F16)
    att_in = nc.dram_tensor("att_in", [128, C], F16)
    att_out = nc.dram_tensor("att_out", [C, C], F16, addr_space="Shared")

    groups8 = [list(range(NCORES))]
    groups4 = [[0, 1, 2, 3], [4, 5, 6, 7]]

    with tile.TileContext(nc) as tc:
        with (
            tc.tile_pool(name="pw", bufs=1) as pw,
            tc.tile_pool(name="psg", bufs=4) as psg,
            tc.tile_pool(name="psc", bufs=2) as psc,
            tc.tile_pool(name="pps", bufs=8, space="PSUM") as pps,
        ):
            # ---- input loads: pairs of 128-row chunks per DMA ---------------
            def load8(dram, width, tag):
                ts = []
                for a in range(4):
                    big = pw.tile([128, 2, width], F16, tag=f"{tag}{a}")
                    nc.sync.dma_start(
                        big[:],
                        dram[256 * a:256 * (a + 1), :].rearrange(
                            "(c p) w -> p c w", c=2
                        ),
                    )
                    ts.append(big[:, 0, :])
                    ts.append(big[:, 1, :])
                return ts

            # ---- local transposed projections -------------------------------
            # proj(X, WT, b)[s, o] = sum_c X[c, s] WT[c, o] + b[o]  -> [512, 512]
            # result stays in SBUF as 4 [128, 512] fp16 tiles (s on partitions).
            def proj(x_tiles, w_tiles, bias_t, otag):
                outs = []
                for ssub in range(4):
                    ps = pps.tile([128, CH], F32, tag="mm")
                    for c in range(8):
                        nc.tensor.matmul(
                            ps[:],
                            x_tiles[c][:, 128 * ssub:128 * (ssub + 1)],
                            w_tiles[c][:],
                            start=(c == 0),
                            stop=(c == 7),
                        )
                    o = pw.tile([128, CH], F16, tag=f"{otag}{ssub}")
                    nc.vector.tensor_add(o[:], ps[:], bias_t[:])
                    outs.append(o)
                return outs

            def partials(ck, cq, rin):
                # For o-tile m, t'-half tp: partial[128 o, 512] over local s
                for m in range(4):
                    for tp in range(2):
                        psl = pps.tile([128, CH], F32, tag="mm")
                        for k in range(4):
                            nc.tensor.matmul(
                                psl[:],
                                ck[k][:, 128 * m:128 * (m + 1)],
                                cq[tp][k][:],
                                start=(k == 0),
                                stop=(k == 3),
                            )
                        stg = psg.tile([128, CH], F16, tag="stg")
                        nc.vector.tensor_copy(stg[:], psl[:])
                        nc.sync.dma_start(
                            rin[128 * m:128 * (m + 1), CH * tp:CH * (tp + 1)],
                            stg[:],
                        )

            # rs_out holds the 4-way summed logits for my 128 attention rows;
            # softmax along the free dim.
            def softmax_from_rs(rs_out, slot, btag):
                lg = pw.tile([128, C], F16, tag=f"{btag}lg")
                nc.sync.dma_start(lg[:], rs_out[:, :])
                mxn = psc.tile([128, 1], F32, tag="mx")
                nc.vector.reduce_max(
                    mxn[:], lg[:], axis=mybir.AxisListType.X, negate=True
                )
                sm = psc.tile([128, 1], F32, tag="sm")
                elg = pw.tile([128, C], F16, tag=f"{btag}el")
                nc.scalar.activation(
                    elg[:],
                    lg[:],
                    mybir.ActivationFunctionType.Exp,
                    bias=mxn[:, 0:1],
                    accum_out=sm[:, 0:1],
                )
                rcp = psc.tile([128, 1], F32, tag="rc")
                nc.vector.reciprocal(rcp[:], sm[:])
                at = pw.tile([128, C], F16, tag=slot)
                nc.vector.tensor_scalar_mul(at[:], elg[:], rcp[:, 0:1])
                return at

            # ---- f-branch chain ---------------------------------------------
            xf_t = load8(xf, S, "xf")
            wk1_t = load8(wk1, CH, "wk1")
            bias_t = pw.tile([128, CH], F32, tag="bk1")
            nc.sync.dma_start(bias_t[:], bk1r[:, :])
            ckf = proj(xf_t, wk1_t, bias_t, "ckf")

            wq_t = load8(wq, CH, "wq")
            bq_t = pw.tile([128, CH], F32, tag="bq")
            nc.sync.dma_start(bq_t[:], bqr[:, :])
            xq0_t = load8(xq0, S, "xf")              # reuse xf slots (dead)
            cq0 = proj(xq0_t, wq_t, bq_t, "cq0")
            xq1_t = load8(xq1, S, "xl")
            cq1 = proj(xq1_t, wq_t, bq_t, "cq1")
            cq = [cq0, cq1]

            partials(ckf, cq, rs_f_in)
            nc.gpsimd.collective_compute(
                "ReduceScatter",
                mybir.AluOpType.add,
                ins=[rs_f_in[:]],
                outs=[rs_f_out[:]],
                replica_groups=groups4,
            )

            # ---- l-branch chain (overlaps the f ReduceScatter) --------------
            xl_t = load8(xl, S, "xl")                # reuse xq1 slots (dead)
            wk2_t = load8(wk2, CH, "wk2")
            bias2_t = pw.tile([128, CH], F32, tag="bk2")
            nc.sync.dma_start(bias2_t[:], bk2r[:, :])
            ckl = proj(xl_t, wk2_t, bias2_t, "ckl")

            partials(ckl, cq, rs_l_in)
            nc.gpsimd.collective_compute(
                "ReduceScatter",
                mybir.AluOpType.add,
                ins=[rs_l_in[:]],
                outs=[rs_l_out[:]],
                replica_groups=groups4,
            )

            # ---- f softmax during the l ReduceScatter window ----------------
            at_f = softmax_from_rs(rs_f_out, "wq0", "af")

            # ---- V projection (local): V[J', hw_d] in fp16, bias per J' ------
            xm_t = load8(xm, S, "xm")
            wv_t = load8(wv, C, "wv")
            bv_t = pw.tile([128, 8], F32, tag="bv")
            nc.sync.dma_start(bv_t[:], bvp[:, :])
            v_sb = []
            for j in range(8):
                ps = pps.tile([128, S], F32, tag="mm")
                for c in range(8):
                    nc.tensor.matmul(
                        ps[:],
                        wv_t[c][:, 128 * j:128 * (j + 1)],
                        xm_t[c][:],
                        start=(c == 0),
                        stop=(c == 7),
                    )
                v = pw.tile([128, S], F16, tag=f"v{j}")
                nc.vector.tensor_scalar_add(v[:], ps[:], bv_t[:, j:j + 1])
                v_sb.append(v)

            # ---- l softmax, attention sum, AllGather halves -----------------
            at_l = softmax_from_rs(rs_l_out, "wq1", "al")
            att_sum = pw.tile([128, C], F16, tag="wq2")
            nc.vector.tensor_add(att_sum[:], at_f[:], at_l[:])
            nc.sync.dma_start(att_in[:, 0:CH], att_sum[:, 0:CH])
            nc.sync.dma_start(att_in[:, CH:C], att_sum[:, CH:C])
            nc.gpsimd.collective_compute(
                "AllGather",
                mybir.AluOpType.bypass,
                ins=[att_in[:]],
                outs=[att_out[:]],
                replica_groups=groups8,
            )

            # ---- residual: R[e] = 0.5 * (xf + xl) on permuted rows -----------
            # Permuted row J' = 512t + o maps to source row j = 2o + t, read
            # straight from the fp16 x tensors with a strided view.
            xf_v = xf[:].rearrange("(o t) w -> t o w", t=2)
            xl_v = xl[:].rearrange("(o t) w -> t o w", t=2)
            r_sb = []

            def resid(e):
                a = pw.tile([128, S], F16, tag=f"wk1{e % 4}", name=f"ra{e}")
                nc.sync.dma_start(
                    a[:], xf_v[e // 4, 128 * (e % 4):128 * (e % 4 + 1), :]
                )
                b = pw.tile([128, S], F16, tag=f"wk2{e % 4}", name=f"rb{e}")
                nc.sync.dma_start(
                    b[:], xl_v[e // 4, 128 * (e % 4):128 * (e % 4 + 1), :]
                )
                r = pw.tile([128, S], F32, tag=f"r{e}", name=f"r{e}")
                nc.vector.tensor_add(r[:], a[:], b[:])
                nc.scalar.mul(r[:], r[:], 0.5)
                r_sb.append(r)

            for e in range(8):
                resid(e)

            # ---- out[:, hw_d] = att @ V_d + R -------------------------------
            att_t = []
            for k in range(8):
                t = pw.tile(
                    [128, C],
                    F16,
                    tag=f"xm{k % 4}" if k < 4 else f"xf{k % 4}",
                    name=f"att_t{k}",
                )
                nc.sync.dma_start(
                    t[:], att_out[:, 128 * k:128 * (k + 1)], transpose=True
                )
                att_t.append(t)
            out_v = out_ext[:].rearrange("(o t) w -> t o w", t=2)
            pse = [
                pps.tile([128, S], F32, tag="mm", name=f"pse{e}") for e in range(8)
            ]
            for k in range(7):
                for e in range(8):
                    nc.tensor.matmul(
                        pse[e][:],
                        att_t[k][:, 128 * e:128 * (e + 1)],
                        v_sb[k][:],
                        start=(k == 0),
                        stop=False,
                    )
            for e in range(8):
                nc.tensor.matmul(
                    pse[e][:],
                    att_t[7][:, 128 * e:128 * (e + 1)],
                    v_sb[7][:],
                    start=False,
                    stop=True,
                )
                ost = pw.tile([128, S], F32, tag=f"o{e % 2}")
                nc.vector.tensor_add(ost[:], pse[e][:], r_sb[e][:])
                nc.sync.dma_start(
                    out_v[e // 4, 128 * (e % 4):128 * (e % 4 + 1), :], ost[:]
                )

    nc.compile()
    return nc


def _prep_inputs(x_f, x_m, x_l, Wq, bq, Wk1, bk1, Wk2, bk2, Wv, bv, gamma):
    Xf = np.ascontiguousarray(x_f.reshape(C, HW), dtype=np.float16)
    Xm = np.ascontiguousarray(x_m.reshape(C, HW), dtype=np.float16)
    Xl = np.ascontiguousarray(x_l.reshape(C, HW), dtype=np.float16)
    g = np.float32(np.asarray(gamma).reshape(-1)[0])

    permJ = 2 * (np.arange(C) % 512) + np.arange(C) // 512  # J' -> global j
    wv_full = np.ascontiguousarray((g * Wv)[permJ, :].T, dtype=np.float16)
    bv_perm = (g * bv)[permJ].astype(np.float32)

    wq_full = np.ascontiguousarray(Wq.T, dtype=np.float16)
    wk1_full = np.ascontiguousarray(Wk1.T, dtype=np.float16)
    wk2_full = np.ascontiguousarray(Wk2.T, dtype=np.float16)
    bqr = np.ascontiguousarray(np.broadcast_to(bq, (128, CH)), dtype=np.float32)
    bk1r = np.ascontiguousarray(np.broadcast_to(bk1, (128, CH)), dtype=np.float32)
    bk2r = np.ascontiguousarray(np.broadcast_to(bk2, (128, CH)), dtype=np.float32)
    bvp = np.ascontiguousarray(bv_perm.reshape(8, 128).T)

    in_maps = []
    for d in range(NCORES):
        sl = slice(S * d, S * (d + 1))
        s0 = slice(S * (d % 4), S * (d % 4 + 1))
        s1 = slice(S * (4 + d % 4), S * (4 + d % 4 + 1))
        in_maps.append({
            "xm": np.ascontiguousarray(Xm[:, sl]),
            "xf": np.ascontiguousarray(Xf[:, sl]),
            "xl": np.ascontiguousarray(Xl[:, sl]),
            "xq0": np.ascontiguousarray(Xm[:, s0]),
            "xq1": np.ascontiguousarray(Xm[:, s1]),
            "wq": wq_full,
            "wk1": wk1_full,
            "wk2": wk2_full,
            "wv": wv_full,
            "bqr": bqr,
            "bk1r": bk1r,
            "bk2r": bk2r,
            "bvp": bvp,
        })
    return in_maps


def _run(inputs: dict, trace: bool = False, **kw):
    if "nc" not in _CACHE:
        _CACHE["nc"] = _build()
    nc = _CACHE["nc"]
    in_maps = _prep_inputs(**inputs)
    res = run_bass_kernel_spmd(nc, in_maps, list(range(NCORES)), trace=trace, **kw)
    out = np.empty((C, HW), np.float32)
    for d in range(NCORES):
        out[:, S * d:S * (d + 1)] = res.results[d]["out"]
    return out.reshape(1, C, 64, 64), res


def kernel(**inputs) -> np.ndarray:
    inputs = {k: np.asarray(v) for k, v in inputs.items()}
    out, _ = _run(inputs)
    return out


# revision 20
# speedup vs baseline: 1.0832x; 1.0832x over previous
"""CoAtten2 Trainium2 kernel: 8-way tensor-parallel over one TRN2 chip.

Reference computation (C=1024, H=W=64, HW=4096):
    q   = (Wq @ Xm + bq)  viewed [1024, 2048] then transposed
    kf  = (Wk1 @ Xf + bk1) viewed [1024, 2048]
    kl  = (Wk2 @ Xl + bk2) viewed [1024, 2048]
    att = softmax(kf @ q) + softmax(kl @ q)          # [1024, 1024]
    out = gamma * (att @ (Wv @ Xm + bv)) + (Xf + Xl)/2

Decomposition (per core d of 8; group t = d//4, a = d%4):
  - Channel indices are permuted (I' = 512t + o <-> i = 2o + t) so the
    torch-style reshape becomes contiguous; the permutation is folded into the
    host-side Wv/bv prep and the output DMA access pattern; gamma into Wv/bv.
  - logits_PERM splits into parity quadrants Q(t, t') whose kf operand needs
    spatial columns [2048t, 2048t+2048) and whose q operand needs spatial
    columns [2048t', ...). Core d owns spatial slice S_d = [512d, 512(d+1));
    it computes the partial contraction over S_d of Q(t=d//4, t'=0 and 1) for
    both attention branches, using ONLY local projections:
      CkfT_d / CklT_d from its own x slices, and CqT over spatial blocks
      (d%4) and (4 + d%4) from two host-provided Xm slices (xq0/xq1).
  - Per branch, one 4-way fp16 ReduceScatter (groups [0-3], [4-7]) sums the
    partial logits and deals each core its 128-row attention block.
  - softmax is a free-dim reduction; the summed attention is AllGathered in
    fp16; each core then computes its output column slice att @ V_d + the
    residual and writes it out in fp16 (converted to f32 on the host).
  - All data-plane tensors are fp16 (x, weights, partials, att, v); f32 is
    kept for PSUM accumulation, softmax internals, biases, residual and the
    final output.
"""

import sys

sys.path.insert(0, "/opt/trn_rl_repo")

import numpy as np

import concourse.bacc as bacc
import concourse.mybir as mybir
from concourse import tile
from concourse.bass_utils import run_bass_kernel_spmd

F32 = mybir.dt.float32
F16 = mybir.dt.float16

C = 1024
HW = 4096
S = 512          # spatial columns per core
CH = 512         # C // 2 (projection output channels)
NCORES = 8

_CACHE: dict = {}


def _build():
    nc = bacc.Bacc("TRN2", target_bir_lowering=False, debug=False, num_devices=NCORES)

    # per-core external inputs (fp16 data plane, f32 biases)
    xm = nc.declare_dram_parameter("xm", [C, S], F16, isOutput=False)
    xf = nc.declare_dram_parameter("xf", [C, S], F16, isOutput=False)
    xl = nc.declare_dram_parameter("xl", [C, S], F16, isOutput=False)
    xq0 = nc.declare_dram_parameter("xq0", [C, S], F16, isOutput=False)  # Xm block d%4
    xq1 = nc.declare_dram_parameter("xq1", [C, S], F16, isOutput=False)  # Xm block 4+d%4
    wq = nc.declare_dram_parameter("wq", [C, CH], F16, isOutput=False)   # Wq.T
    wk1 = nc.declare_dram_parameter("wk1", [C, CH], F16, isOutput=False)
    wk2 = nc.declare_dram_parameter("wk2", [C, CH], F16, isOutput=False)
    wv = nc.declare_dram_parameter("wv", [C, C], F16, isOutput=False)    # (g*Wv)[permJ].T
    bqr = nc.declare_dram_parameter("bqr", [128, CH], F32, isOutput=False)
    bk1r = nc.declare_dram_parameter("bk1r", [128, CH], F32, isOutput=False)
    bk2r = nc.declare_dram_parameter("bk2r", [128, CH], F32, isOutput=False)
    bvp = nc.declare_dram_parameter("bvp", [128, 8], F32, isOutput=False)
    out_ext = nc.declare_dram_parameter("out", [C, S], F16, isOutput=True)

    # internal DRAM
    rs_f_in = nc.dram_tensor("rs_f_in", [CH, C], F16)
    rs_f_out = nc.dram_tensor("rs_f_out", [128, C], F16)
    rs_l_in = nc.dram_tensor("rs_l_in", [CH, C], F16)
    rs_l_out = nc.dram_tensor("rs_l_out", [128, C], F16)
    att_in = nc.dram_tensor("att_in", [128, C], F16)
    att_out = nc.dram_tensor("att_out", [C, C], F16, addr_space="Shared")

    groups8 = [list(range(NCORES))]
    groups4 = [[0, 1, 2, 3], [4, 5, 6, 7]]

    with tile.TileContext(nc) as tc:
        with (
            tc.tile_pool(name="pw", bufs=1) as pw,
            tc.tile_pool(name="psg", bufs=4) as psg,
            tc.tile_pool(name="psc", bufs=2) as psc,
            tc.tile_pool(name="pps", bufs=8, space="PSUM") as pps,
        ):
            # ---- input loads: pairs of 128-row chunks per DMA ---------------
            def load8(dram, width, tag):
                ts = []
                for a in range(4):
                    big = pw.tile([128, 2, width], F16, tag=f"{tag}{a}")
                    nc.sync.dma_start(
                        big[:],
                        dram[256 * a:256 * (a + 1), :].rearrange(
                            "(c p) w -> p c w", c=2
                        ),
                    )
                    ts.append(big[:, 0, :])
                    ts.append(big[:, 1, :])
                return ts

            # ---- local transposed projections -------------------------------
            # proj(X, WT, b)[s, o] = sum_c X[c, s] WT[c, o] + b[o]  -> [512, 512]
            # result stays in SBUF as 4 [128, 512] fp16 tiles (s on partitions).
            def proj(x_tiles, w_tiles, bias_t, otag):
                outs = []
                for ssub in range(4):
                    ps = pps.tile([128, CH], F32, tag="mm")
                    for c in range(8):
                        nc.tensor.matmul(
                            ps[:],
                            x_tiles[c][:, 128 * ssub:128 * (ssub + 1)],
                            w_tiles[c][:],
                            start=(c == 0),
                            stop=(c == 7),
                        )
                    o = pw.tile([128, CH], F16, tag=f"{otag}{ssub}")
                    nc.vector.tensor_add(o[:], ps[:], bias_t[:])
                    outs.append(o)
                return outs

            def partials(ck, cq, rin):
                # For o-tile m, t'-half tp: partial[128 o, 512] over local s
                for m in range(4):
                    for tp in range(2):
                        psl = pps.tile([128, CH], F32, tag="mm")
                        for k in range(4):
                            nc.tensor.matmul(
                                psl[:],
                                ck[k][:, 128 * m:128 * (m + 1)],
                                cq[tp][k][:],
                                start=(k == 0),
                                stop=(k == 3),
                            )
                        stg = psg.tile([128, CH], F16, tag="stg")
                        nc.vector.tensor_copy(stg[:], psl[:])
                        nc.sync.dma_start(
                            rin[128 * m:128 * (m + 1), CH * tp:CH * (tp + 1)],
                            stg[:],
                        )

            # rs_out holds the 4-way summed logits for my 128 attention rows;
            # softmax along the free dim.
            def softmax_from_rs(rs_out, slot, btag):
                lg = pw.tile([128, C], F16, tag=f"{btag}lg")
                nc.sync.dma_start(lg[:], rs_out[:, :])
                mxn = psc.tile([128, 1], F32, tag="mx")
                nc.vector.reduce_max(
                    mxn[:], lg[:], axis=mybir.AxisListType.X, negate=True
                )
                sm = psc.tile([128, 1], F32, tag="sm")
                elg = pw.tile([128, C], F16, tag=f"{btag}el")
                nc.scalar.activation(
                    elg[:],
                    lg[:],
                    mybir.ActivationFunctionType.Exp,
                    bias=mxn[:, 0:1],
                    accum_out=sm[:, 0:1],
                )
                rcp = psc.tile([128, 1], F32, tag="rc")
                nc.vector.reciprocal(rcp[:], sm[:])
                at = pw.tile([128, C], F16, tag=slot)
                nc.vector.tensor_scalar_mul(at[:], elg[:], rcp[:, 0:1])
                return at

            # ---- f-branch chain ---------------------------------------------
            xf_t = load8(xf, S, "xf")
            wk1_t = load8(wk1, CH, "wk1")
            bias_t = pw.tile([128, CH], F32, tag="bk1")
            nc.sync.dma_start(bias_t[:], bk1r[:, :])
            ckf = proj(xf_t, wk1_t, bias_t, "ckf")

            wq_t = load8(wq, CH, "wq")
            bq_t = pw.tile([128, CH], F32, tag="bq")
            nc.sync.dma_start(bq_t[:], bqr[:, :])
            xq0_t = load8(xq0, S, "xf")              # reuse xf slots (dead)
            cq0 = proj(xq0_t, wq_t, bq_t, "cq0")
            xq1_t = load8(xq1, S, "xl")
            cq1 = proj(xq1_t, wq_t, bq_t, "cq1")
            cq = [cq0, cq1]

            partials(ckf, cq, rs_f_in)
            nc.gpsimd.collective_compute(
                "ReduceScatter",
                mybir.AluOpType.add,
                ins=[rs_f_in[:]],
                outs=[rs_f_out[:]],
                replica_groups=groups4,
            )

            # ---- l-branch chain (overlaps the f ReduceScatter) --------------
            xl_t = load8(xl, S, "xl")                # reuse xq1 slots (dead)
            wk2_t = load8(wk2, CH, "wk2")
            bias2_t = pw.tile([128, CH], F32, tag="bk2")
            nc.sync.dma_start(bias2_t[:], bk2r[:, :])
            ckl = proj(xl_t, wk2_t, bias2_t, "ckl")

            partials(ckl, cq, rs_l_in)
            nc.gpsimd.collective_compute(
                "ReduceScatter",
                mybir.AluOpType.add,
                ins=[rs_l_in[:]],
                outs=[rs_l_out[:]],
                replica_groups=groups4,
            )

            # ---- f softmax during the l ReduceScatter window ----------------
            at_f = softmax_from_rs(rs_f_out, "wq0", "af")

            # ---- V projection (local): V[J', hw_d] in fp16, bias per J' ------
            xm_t = load8(xm, S, "xm")
            wv_t = load8(wv, C, "wv")
            bv_t = pw.tile([128, 8], F32, tag="bv")
            nc.sync.dma_start(bv_t[:], bvp[:, :])
            v_sb = []
            for j in range(8):
                ps = pps.tile([128, S], F32, tag="mm")
                for c in range(8):
                    nc.tensor.matmul(
                        ps[:],
                        wv_t[c][:, 128 * j:128 * (j + 1)],
                        xm_t[c][:],
                        start=(c == 0),
                        stop=(c == 7),
                    )
                v = pw.tile([128, S], F16, tag=f"v{j}")
                nc.vector.tensor_scalar_add(v[:], ps[:], bv_t[:, j:j + 1])
                v_sb.append(v)

            # ---- l softmax, attention sum, AllGather halves -----------------
            at_l = softmax_from_rs(rs_l_out, "wq1", "al")
            att_sum = pw.tile([128, C], F16, tag="wq2")
            nc.vector.tensor_add(att_sum[:], at_f[:], at_l[:])
            nc.sync.dma_start(att_in[:, 0:CH], att_sum[:, 0:CH])
            nc.sync.dma_start(att_in[:, CH:C], att_sum[:, CH:C])
            nc.gpsimd.collective_compute(
                "AllGather",
                mybir.AluOpType.bypass,
                ins=[att_in[:]],
                outs=[att_out[:]],
                replica_groups=groups8,
            )

            # ---- residual: R[e] = 0.5 * (xf + xl) on permuted rows -----------
            # Permuted row J' = 512t + o maps to source row j = 2o + t, read
            # straight from the fp16 x tensors with a strided view.
            xf_v = xf[:].rearrange("(o t) w -> t o w", t=2)
            xl_v = xl[:].rearrange("(o t) w -> t o w", t=2)
            r_sb = []

            def resid(e):
                a = pw.tile([128, S], F16, tag=f"wk1{e % 4}", name=f"ra{e}")
                nc.sync.dma_start(
                    a[:], xf_v[e // 4, 128 * (e % 4):128 * (e % 4 + 1), :]
                )
                b = pw.tile([128, S], F16, tag=f"wk2{e % 4}", name=f"rb{e}")
                nc.sync.dma_start(
                    b[:], xl_v[e // 4, 128 * (e % 4):128 * (e % 4 + 1), :]
                )
                r = pw.tile([128, S], F32, tag=f"r{e}", name=f"r{e}")
                nc.vector.tensor_add(r[:], a[:], b[:])
                nc.scalar.mul(r[:], r[:], 0.5)
                r_sb.append(r)

            for e in range(8):
                resid(e)

            # ---- out[:, hw_d] = att @ V_d + R -------------------------------
            att_t = []
            for k in range(8):
                t = pw.tile(
                    [128, C],
                    F16,
                    tag=f"xm{k % 4}" if k < 4 else f"xf{k % 4}",
                    name=f"att_t{k}",
                )
                nc.sync.dma_start(
                    t[:], att_out[:, 128 * k:128 * (k + 1)], transpose=True
                )
                att_t.append(t)
            out_v = out_ext[:].rearrange("(o t) w -> t o w", t=2)
            for e in range(8):
                ps = pps.tile([128, S], F32, tag="mm")
                for k in range(8):
                    nc.tensor.matmul(
                        ps[:],
                        att_t[k][:, 128 * e:128 * (e + 1)],
                        v_sb[k][:],
                        start=(k == 0),
                        stop=(k == 7),
                    )
                ost = pw.tile([128, S], F16, tag=f"o{e % 2}")
                nc.vector.tensor_add(ost[:], ps[:], r_sb[e][:])
                nc.sync.dma_start(
                    out_v[e // 4, 128 * (e % 4):128 * (e % 4 + 1), :], ost[:]
                )

    nc.compile()
    return nc


def _prep_inputs(x_f, x_m, x_l, Wq, bq, Wk1, bk1, Wk2, bk2, Wv, bv, gamma):
    Xf = np.ascontiguousarray(x_f.reshape(C, HW), dtype=np.float16)
    Xm = np.ascontiguousarray(x_m.reshape(C, HW), dtype=np.float16)
    Xl = np.ascontiguousarray(x_l.reshape(C, HW), dtype=np.float16)
    g = np.float32(np.asarray(gamma).reshape(-1)[0])

    permJ = 2 * (np.arange(C) % 512) + np.arange(C) // 512  # J' -> global j
    wv_full = np.ascontiguousarray((g * Wv)[permJ, :].T, dtype=np.float16)
    bv_perm = (g * bv)[permJ].astype(np.float32)

    wq_full = np.ascontiguousarray(Wq.T, dtype=np.float16)
    wk1_full = np.ascontiguousarray(Wk1.T, dtype=np.float16)
    wk2_full = np.ascontiguousarray(Wk2.T, dtype=np.float16)
    bqr = np.ascontiguousarray(np.broadcast_to(bq, (128, CH)), dtype=np.float32)
    bk1r = np.ascontiguousarray(np.broadcast_to(bk1, (128, CH)), dtype=np.float32)
    bk2r = np.ascontiguousarray(np.broadcast_to(bk2, (128, CH)), dtype=np.float32)
    bvp = np.ascontiguousarray(bv_perm.reshape(8, 128).T)

    in_maps = []
    for d in range(NCORES):
        sl = slice(S * d, S * (d + 1))
        s0 = slice(S * (d % 4), S * (d % 4 + 1))
        s1 = slice(S * (4 + d % 4), S * (4 + d % 4 + 1))
        in_maps.append({
            "xm": np.ascontiguousarray(Xm[:, sl]),
            "xf": np.ascontiguousarray(Xf[:, sl]),
            "xl": np.ascontiguousarray(Xl[:, sl]),
            "xq0": np.ascontiguousarray(Xm[:, s0]),
            "xq1": np.ascontiguousarray(Xm[:, s1]),
            "wq": wq_full,
            "wk1": wk1_full,
            "wk2": wk2_full,
            "wv": wv_full,
            "bqr": bqr,
            "bk1r": bk1r,
            "bk2r": bk2r,
            "bvp": bvp,
        })
    return in_maps


def _run(inputs: dict, trace: bool = False, **kw):
    if "nc" not in _CACHE:
        _CACHE["nc"] = _build()
    nc = _CACHE["nc"]
    in_maps = _prep_inputs(**inputs)
    res = run_bass_kernel_spmd(nc, in_maps, list(range(NCORES)), trace=trace, **kw)
    out = np.empty((C, HW), np.float32)
    for d in range(NCORES):
        out[:, S * d:S * (d + 1)] = res.results[d]["out"].astype(np.float32)
    return out.reshape(1, C, 64, 64), res


def kernel(**inputs) -> np.ndarray:
    inputs = {k: np.asarray(v) for k, v in inputs.items()}
    out, _ = _run(inputs)
    return out


# revision 21
# speedup vs baseline: 1.1775x; 1.0871x over previous
"""CoAtten2 Trainium2 kernel: 8-way tensor-parallel over one TRN2 chip.

Reference computation (C=1024, H=W=64, HW=4096):
    q   = (Wq @ Xm + bq)  viewed [1024, 2048] then transposed
    kf  = (Wk1 @ Xf + bk1) viewed [1024, 2048]
    kl  = (Wk2 @ Xl + bk2) viewed [1024, 2048]
    att = softmax(kf @ q) + softmax(kl @ q)          # [1024, 1024]
    out = gamma * (att @ (Wv @ Xm + bv)) + (Xf + Xl)/2

Decomposition (per core d of 8; group t = d//4, a = d%4):
  - Channel indices are permuted (I' = 512t + o <-> i = 2o + t) so the
    torch-style reshape becomes contiguous; the permutation is folded into the
    host-side Wv/bv prep and the output DMA access pattern; gamma into Wv/bv.
  - logits_PERM splits into parity quadrants Q(t, t') whose kf operand needs
    spatial columns [2048t, 2048t+2048) and whose q operand needs spatial
    columns [2048t', ...). Core d owns spatial slice S_d = [512d, 512(d+1));
    it computes the partial contraction over S_d of Q(t=d//4, t'=0 and 1) for
    both attention branches, using ONLY local projections:
      CkfT_d / CklT_d from its own x slices, and CqT over spatial blocks
      (d%4) and (4 + d%4) from two host-provided Xm slices (xq0/xq1).
  - Per branch, one 4-way fp16 ReduceScatter (groups [0-3], [4-7]) sums the
    partial logits and deals each core its 128-row attention block.
  - softmax is a free-dim reduction; the summed attention is AllGathered in
    fp16; each core then computes its output column slice att @ V_d + the
    f32 residual.
  - All data-plane tensors are fp16 (x, weights, partials, att, v); f32 is
    kept for PSUM accumulation, softmax internals, biases, residual and the
    final output.
"""

import sys

sys.path.insert(0, "/opt/trn_rl_repo")

import numpy as np

import concourse.bacc as bacc
import concourse.mybir as mybir
from concourse import tile
from concourse.bass_utils import run_bass_kernel_spmd

F32 = mybir.dt.float32
F16 = mybir.dt.float16

C = 1024
HW = 4096
S = 512          # spatial columns per core
CH = 512         # C // 2 (projection output channels)
NCORES = 8

_CACHE: dict = {}


def _build():
    nc = bacc.Bacc("TRN2", target_bir_lowering=False, debug=False, num_devices=NCORES)

    # per-core external inputs (fp16 data plane, f32 biases)
    xm = nc.declare_dram_parameter("xm", [C, S], F16, isOutput=False)
    xf = nc.declare_dram_parameter("xf", [C, S], F16, isOutput=False)
    xl = nc.declare_dram_parameter("xl", [C, S], F16, isOutput=False)
    xq0 = nc.declare_dram_parameter("xq0", [C, S], F16, isOutput=False)  # Xm block d%4
    xq1 = nc.declare_dram_parameter("xq1", [C, S], F16, isOutput=False)  # Xm block 4+d%4
    wq = nc.declare_dram_parameter("wq", [C, CH], F16, isOutput=False)   # Wq.T
    wk1 = nc.declare_dram_parameter("wk1", [C, CH], F16, isOutput=False)
    wk2 = nc.declare_dram_parameter("wk2", [C, CH], F16, isOutput=False)
    wv = nc.declare_dram_parameter("wv", [C, C], F16, isOutput=False)    # (g*Wv)[permJ].T
    bqr = nc.declare_dram_parameter("bqr", [128, CH], F32, isOutput=False)
    bk1r = nc.declare_dram_parameter("bk1r", [128, CH], F32, isOutput=False)
    bk2r = nc.declare_dram_parameter("bk2r", [128, CH], F32, isOutput=False)
    bvp = nc.declare_dram_parameter("bvp", [128, 8], F32, isOutput=False)
    out_ext = nc.declare_dram_parameter("out", [C, S], F32, isOutput=True)

    # internal DRAM
    rs_f_in = nc.dram_tensor("rs_f_in", [CH, C], F16)
    rs_f_out = nc.dram_tensor("rs_f_out", [128, C], F16)
    rs_l_in = nc.dram_tensor("rs_l_in", [CH, C], F16)
    rs_l_out = nc.dram_tensor("rs_l_out", [128, C], F16)
    att_in = nc.dram_tensor("att_in", [128, C], F16)
    att_out = nc.dram_tensor("att_out", [C, C], F16, addr_space="Shared")

    groups8 = [list(range(NCORES))]
    groups4 = [[0, 1, 2, 3], [4, 5, 6, 7]]

    with tile.TileContext(nc) as tc:
        with (
            tc.tile_pool(name="pw", bufs=1) as pw,
            tc.tile_pool(name="psg", bufs=4) as psg,
            tc.tile_pool(name="psc", bufs=2) as psc,
            tc.tile_pool(name="pps", bufs=8, space="PSUM") as pps,
        ):
            # ---- input loads: pairs of 128-row chunks per DMA ---------------
            def load8(dram, width, tag):
                ts = []
                for a in range(4):
                    big = pw.tile([128, 2, width], F16, tag=f"{tag}{a}")
                    nc.sync.dma_start(
                        big[:],
                        dram[256 * a:256 * (a + 1), :].rearrange(
                            "(c p) w -> p c w", c=2
                        ),
                    )
                    ts.append(big[:, 0, :])
                    ts.append(big[:, 1, :])
                return ts

            # ---- local transposed projections -------------------------------
            # proj(X, WT, b)[s, o] = sum_c X[c, s] WT[c, o] + b[o]  -> [512, 512]
            # result stays in SBUF as 4 [128, 512] fp16 tiles (s on partitions).
            def proj(x_tiles, w_tiles, bias_t, otag):
                outs = []
                for ssub in range(4):
                    ps = pps.tile([128, CH], F32, tag="mm")
                    for c in range(8):
                        nc.tensor.matmul(
                            ps[:],
                            x_tiles[c][:, 128 * ssub:128 * (ssub + 1)],
                            w_tiles[c][:],
                            start=(c == 0),
                            stop=(c == 7),
                        )
                    o = pw.tile([128, CH], F16, tag=f"{otag}{ssub}")
                    nc.vector.tensor_add(o[:], ps[:], bias_t[:])
                    outs.append(o)
                return outs

            def partials(ck, cq, rin):
                # For o-tile m, t'-half tp: partial[128 o, 512] over local s
                for m in range(4):
                    for tp in range(2):
                        psl = pps.tile([128, CH], F32, tag="mm")
                        for k in range(4):
                            nc.tensor.matmul(
                                psl[:],
                                ck[k][:, 128 * m:128 * (m + 1)],
                                cq[tp][k][:],
                                start=(k == 0),
                                stop=(k == 3),
                            )
                        stg = psg.tile([128, CH], F16, tag="stg")
                        nc.vector.tensor_copy(stg[:], psl[:])
                        nc.sync.dma_start(
                            rin[128 * m:128 * (m + 1), CH * tp:CH * (tp + 1)],
                            stg[:],
                        )

            # rs_out holds the 4-way summed logits for my 128 attention rows;
            # softmax along the free dim.
            def softmax_from_rs(rs_out, slot, btag):
                lg = pw.tile([128, C], F16, tag=f"{btag}lg")
                nc.sync.dma_start(lg[:], rs_out[:, :])
                mxn = psc.tile([128, 1], F32, tag="mx")
                nc.vector.reduce_max(
                    mxn[:], lg[:], axis=mybir.AxisListType.X, negate=True
                )
                sm = psc.tile([128, 1], F32, tag="sm")
                elg = pw.tile([128, C], F16, tag=f"{btag}el")
                nc.scalar.activation(
                    elg[:],
                    lg[:],
                    mybir.ActivationFunctionType.Exp,
                    bias=mxn[:, 0:1],
                    accum_out=sm[:, 0:1],
                )
                rcp = psc.tile([128, 1], F32, tag="rc")
                nc.vector.reciprocal(rcp[:], sm[:])
                at = pw.tile([128, C], F16, tag=slot)
                nc.vector.tensor_scalar_mul(at[:], elg[:], rcp[:, 0:1])
                return at

            # ---- f-branch chain ---------------------------------------------
            xf_t = load8(xf, S, "xf")
            wk1_t = load8(wk1, CH, "wk1")
            bias_t = pw.tile([128, CH], F32, tag="bk1")
            nc.sync.dma_start(bias_t[:], bk1r[:, :])
            ckf = proj(xf_t, wk1_t, bias_t, "ckf")

            wq_t = load8(wq, CH, "wq")
            bq_t = pw.tile([128, CH], F32, tag="bq")
            nc.sync.dma_start(bq_t[:], bqr[:, :])
            xq0_t = load8(xq0, S, "xf")              # reuse xf slots (dead)
            cq0 = proj(xq0_t, wq_t, bq_t, "cq0")
            xq1_t = load8(xq1, S, "xl")
            cq1 = proj(xq1_t, wq_t, bq_t, "cq1")
            cq = [cq0, cq1]

            partials(ckf, cq, rs_f_in)
            nc.gpsimd.collective_compute(
                "ReduceScatter",
                mybir.AluOpType.add,
                ins=[rs_f_in[:]],
                outs=[rs_f_out[:]],
                replica_groups=groups4,
            )

            # ---- l-branch chain (overlaps the f ReduceScatter) --------------
            xl_t = load8(xl, S, "xl")                # reuse xq1 slots (dead)
            wk2_t = load8(wk2, CH, "wk2")
            bias2_t = pw.tile([128, CH], F32, tag="bk2")
            nc.sync.dma_start(bias2_t[:], bk2r[:, :])
            ckl = proj(xl_t, wk2_t, bias2_t, "ckl")

            partials(ckl, cq, rs_l_in)
            nc.gpsimd.collective_compute(
                "ReduceScatter",
                mybir.AluOpType.add,
                ins=[rs_l_in[:]],
                outs=[rs_l_out[:]],
                replica_groups=groups4,
            )

            # ---- f softmax during the l ReduceScatter window ----------------
            at_f = softmax_from_rs(rs_f_out, "wq0", "af")

            # ---- V projection (local): V[J', hw_d] in fp16, bias per J' ------
            xm_t = load8(xm, S, "xm")
            wv_t = load8(wv, C, "wv")
            bv_t = pw.tile([128, 8], F32, tag="bv")
            nc.sync.dma_start(bv_t[:], bvp[:, :])
            v_sb = []
            for j in range(8):
                ps = pps.tile([128, S], F32, tag="mm")
                for c in range(8):
                    nc.tensor.matmul(
                        ps[:],
                        wv_t[c][:, 128 * j:128 * (j + 1)],
                        xm_t[c][:],
                        start=(c == 0),
                        stop=(c == 7),
                    )
                v = pw.tile([128, S], F16, tag=f"v{j}")
                nc.vector.tensor_scalar_add(v[:], ps[:], bv_t[:, j:j + 1])
                v_sb.append(v)

            # ---- l softmax, attention sum, AllGather halves -----------------
            at_l = softmax_from_rs(rs_l_out, "wq1", "al")
            att_sum = pw.tile([128, C], F16, tag="wq2")
            nc.vector.tensor_add(att_sum[:], at_f[:], at_l[:])
            nc.sync.dma_start(att_in[:, 0:CH], att_sum[:, 0:CH])
            nc.sync.dma_start(att_in[:, CH:C], att_sum[:, CH:C])
            nc.gpsimd.collective_compute(
                "AllGather",
                mybir.AluOpType.bypass,
                ins=[att_in[:]],
                outs=[att_out[:]],
                replica_groups=groups8,
            )

            # ---- residual: R[e] = 0.5 * (xf + xl) on permuted rows -----------
            # Permuted row J' = 512t + o maps to source row j = 2o + t, read
            # straight from the fp16 x tensors with a strided view.
            xf_v = xf[:].rearrange("(o t) w -> t o w", t=2)
            xl_v = xl[:].rearrange("(o t) w -> t o w", t=2)
            r_sb = []

            def resid(e):
                a = pw.tile([128, S], F16, tag=f"wk1{e % 4}", name=f"ra{e}")
                nc.sync.dma_start(
                    a[:], xf_v[e // 4, 128 * (e % 4):128 * (e % 4 + 1), :]
                )
                b = pw.tile([128, S], F16, tag=f"wk2{e % 4}", name=f"rb{e}")
                nc.sync.dma_start(
                    b[:], xl_v[e // 4, 128 * (e % 4):128 * (e % 4 + 1), :]
                )
                r = pw.tile([128, S], F32, tag=f"r{e}", name=f"r{e}")
                nc.vector.tensor_add(r[:], a[:], b[:])
                nc.scalar.mul(r[:], r[:], 0.5)
                r_sb.append(r)

            for e in range(8):
                resid(e)

            # ---- out[:, hw_d] = att @ V_d + R -------------------------------
            att_t = []
            for k in range(8):
                t = pw.tile(
                    [128, C],
                    F16,
                    tag=f"xm{k % 4}" if k < 4 else f"xf{k % 4}",
                    name=f"att_t{k}",
                )
                nc.sync.dma_start(
                    t[:], att_out[:, 128 * k:128 * (k + 1)], transpose=True
                )
                att_t.append(t)
            out_v = out_ext[:].rearrange("(o t) w -> t o w", t=2)
            for e in range(8):
                ps = pps.tile([128, S], F32, tag="mm")
                for k in range(8):
                    nc.tensor.matmul(
                        ps[:],
                        att_t[k][:, 128 * e:128 * (e + 1)],
                        v_sb[k][:],
                        start=(k == 0),
                        stop=(k == 7),
                    )
                ost = pw.tile([128, S], F32, tag=f"o{e % 2}")
                nc.vector.tensor_add(ost[:], ps[:], r_sb[e][:])
                nc.sync.dma_start(
                    out_v[e // 4, 128 * (e % 4):128 * (e % 4 + 1), :], ost[:]
                )

    nc.compile()
    return nc


def _prep_inputs(x_f, x_m, x_l, Wq, bq, Wk1, bk1, Wk2, bk2, Wv, bv, gamma):
    Xf = np.ascontiguousarray(x_f.reshape(C, HW), dtype=np.float16)
    Xm = np.ascontiguousarray(x_m.reshape(C, HW), dtype=np.float16)
    Xl = np.ascontiguousarray(x_l.reshape(C, HW), dtype=np.float16)
    g = np.float32(np.asarray(gamma).reshape(-1)[0])

    permJ = 2 * (np.arange(C) % 512) + np.arange(C) // 512  # J' -> global j
    wv_full = np.ascontiguousarray((g * Wv)[permJ, :].T, dtype=np.float16)
    bv_perm = (g * bv)[permJ].astype(np.float32)

    wq_full = np.ascontiguousarray(Wq.T, dtype=np.float16)
    wk1_full = np.ascontiguousarray(Wk1.T, dtype=np.float16)
    wk2_full = np.ascontiguousarray(Wk2.T, dtype=np.float16)
    bqr = np.ascontiguousarray(np.broadcast_to(bq, (128, CH)), dtype=np.float32)
    bk1r = np.ascontiguousarray(np.broadcast_to(bk1, (128, CH)), dtype=np.float32)
    bk2r = np.ascontiguousarray(np.broadcast_to(bk2, (128, CH)), dtype=np.float32)
    bvp = np.ascontiguousarray(bv_perm.reshape(8, 128).T)

    in_maps = []
    for d in range(NCORES):
        sl = slice(S * d, S * (d + 1))
        s0 = slice(S * (d % 4), S * (d % 4 + 1))
        s1 = slice(S * (4 + d % 4), S * (4 + d % 4 + 1))
        in_maps.append({
            "xm": np.ascontiguousarray(Xm[:, sl]),
            "xf": np.ascontiguousarray(Xf[:, sl]),
            "xl": np.ascontiguousarray(Xl[:, sl]),
            "xq0": np.ascontiguousarray(Xm[:, s0]),
            "xq1": np.ascontiguousarray(Xm[:, s1]),
            "wq": wq_full,
            "wk1": wk1_full,
            "wk2": wk2_full,
            "wv": wv_full,
            "bqr": bqr,
            "bk1r": bk1r,
            "bk2r": bk2r,
            "bvp": bvp,
        })
    return in_maps


def _run(inputs: dict, trace: bool = False, **kw):
    if "nc" not in _CACHE:
        _CACHE["nc"] = _build()
    nc = _CACHE["nc"]
    in_maps = _prep_inputs(**inputs)
    res = run_bass_kernel_spmd(nc, in_maps, list(range(NCORES)), trace=trace, **kw)
    out = np.empty((C, HW), np.float32)
    for d in range(NCORES):
        out[:, S * d:S * (d + 1)] = res.results[d]["out"]
    return out.reshape(1, C, 64, 64), res


def kernel(**inputs) -> np.ndarray:
    inputs = {k: np.asarray(v) for k, v in inputs.items()}
    out, _ = _run(inputs)
    return out
